# revision 7
# baseline (speedup 1.0000x reference)
"""Trainium2 Bass kernel for nn_BezierButtress (Bernstein-basis permutation chains).

Math (per permutation chain p, over depth d = 0..31):
    S_mean <- (S_mean @ Wm_d) * B(x_{perm[p,d]})        (K=17 wide state)
    S_var  <- (S_var  @ Wv_d) * B(x_{perm[p,d]})^2
    outputs: f_mean[n] = sum_{p,k} S_mean, f_var[n] = sum_{p,k} S_var / post_prec[p]

Device strategy (data-parallel over N across 8 cores, 3072 rows each):
  * state layout: (7 chains x 17 k = 119 partitions, n free), block-diagonal
    119x119 fp32r chain matmuls (3 groups cover 20 chains).
  * per-step Bernstein multipliers built in log space: one PE matmul contracts
    a baked selection/coefficient matrix A_{d,g} (128 x 119) against a resident
    log-table UV (U_hi/V_hi/U_lo/V_lo, 128 x n) giving
    logM = k*log(x_c) + (16-k)*log(1-x_c) exactly (hi/lo splitting cancels the
    PE fp22 truncation); then ACT computes exp(logM + log binom) and
    exp(2*logM + 2*log binom) (the squared multiplier) -- GPSIMD computes the
    square for half the tiles to offload ACT.
  * meanw0 / exp(varw0)*sc2 / sc2 column scale / 1/post_prec are all folded
    host-side into the baked block-diagonal weights & reduction vectors.
"""

import os
import numpy as np
from math import comb

import concourse.bass as bass
import concourse.mybir as mybir
import concourse.tile as tile
from concourse import bacc
from concourse import bass_utils

ORDER = 16
K = 17
D = 32
P = 20
N = 24576
NCORES = 8
NLOC = N // NCORES        # 3072
CPG = 7                   # chain slots per group
G = 3                     # groups (7, 7, 6 + 1 pad)
R = CPG * K               # 119 partitions
CHUNK = 1024
HALF = 512
F32 = mybir.dt.float32
F32R = mybir.dt.float32r
EXP = mybir.ActivationFunctionType.Exp
MULT = mybir.AluOpType.mult

# fraction of squared-multiplier tiles computed on GPSIMD instead of ACT
GP_EVERY = 2  # every 2nd tile -> 50%


def _fp22_split(x64):
    """Split float64 -> (hi, lo) float32 with hi exactly representable in
    fp22 (e8m13), so PE fp32r matmuls consume hi/lo exactly."""
    x32 = x64.astype(np.float32)
    hi = (x32.view(np.uint32) & np.uint32(0xFFFFFC00)).view(np.float32)
    lo = (x64 - hi.astype(np.float64)).astype(np.float32)
    return hi, lo


def _fp22_round(x64):
    """Round float64 to the nearest fp22 (e8m13) value, returned as float32.
    The PE's fp32r path *truncates* inputs to fp22; feeding it pre-rounded
    values makes that truncation a no-op and kills the systematic bias."""
    x32 = x64.astype(np.float32)
    u = x32.view(np.uint32).astype(np.uint64)
    u = ((u + 0x200) & 0xFFFFFC00).astype(np.uint32)   # round-half-up on m13
    return u.view(np.float32)


# Each chain step's state is stored as fp32 and truncated to fp22 when the
# PE streams it: expected relative loss ~2^-15 per step.  Compensate by
# scaling every per-step multiplier by (1 + 2^-15) (opt-out: BB_BIASCOMP=0).
_STATE_TRUNC_COMP = float(np.log1p(2.0 ** -15))


def _host_tensors(Xnew, meanw0, meanw, varw0, varw, prior_sc, post_prec, perm):
    Xnew = np.asarray(Xnew, np.float32)
    meanw0 = np.asarray(meanw0, np.float64)   # (P, 1, K)
    meanw = np.asarray(meanw, np.float64)     # (D-1, P, K, K)
    varw0 = np.asarray(varw0, np.float64)     # (P, 1, K)
    varw = np.asarray(varw, np.float64)       # (D-1, P, K, K)
    prior_sc = np.asarray(prior_sc, np.float64)  # (K, 1)
    post_prec = np.asarray(post_prec, np.float64)  # (P,)
    perm = np.asarray(perm)                   # (P, D) int

    # --- per-core UV log tables ---------------------------------------
    x64 = np.clip(Xnew.astype(np.float64), 1e-30, None)
    u64 = np.log(x64)                                    # (N, D)
    v64 = np.log1p(-np.minimum(Xnew.astype(np.float64), 1.0 - 1e-15))
    uh, ul = _fp22_split(u64)
    vh, vl = _fp22_split(v64)
    uv_full = np.concatenate(
        [uh.T[None], vh.T[None], ul.T[None], vl.T[None]], axis=0
    )  # (4, D, N)
    uv_shards = []
    for i in range(NCORES):
        sl = uv_full[:, :, i * NLOC:(i + 1) * NLOC]      # (4, D, NLOC)
        uv_shards.append(np.ascontiguousarray(sl.reshape(4 * D, NLOC), np.float32))

    # --- A selection/coefficient matrices (D*G, 128, R) ---------------
    ks = np.arange(K, dtype=np.float64)
    amat = np.zeros((D * G, 4 * D, R), np.float64)
    for d in range(D):
        for g in range(G):
            A = amat[d * G + g]
            for c in range(CPG):
                p = g * CPG + c
                if p >= P:
                    continue
                col = perm[p, d]
                j = slice(K * c, K * c + K)
                A[col, j] = ks
                A[D + col, j] = ORDER - ks
                A[2 * D + col, j] = ks
                A[3 * D + col, j] = ORDER - ks
    amat = amat.astype(np.float32)

    # --- block-diagonal chain weights ---------------------------------
    sc2 = prior_sc[:, 0] ** 2                            # (K,)
    wmean = np.zeros(((D - 1) * G, R, R), np.float64)
    wvar = np.zeros(((D - 1) * G, R, R), np.float64)
    for d in range(1, D):
        for g in range(G):
            Wm = wmean[(d - 1) * G + g]
            Wv = wvar[(d - 1) * G + g]
            for c in range(CPG):
                p = g * CPG + c
                if p >= P:
                    continue
                blk = slice(K * c, K * c + K)
                m = meanw[d - 1, p]                      # (K, K) [k, j]
                v = np.exp(varw[d - 1, p]) * sc2[None, :]
                if d == 1:
                    m = meanw0[p, 0][:, None] * m
                    v = (np.exp(varw0[p, 0]) * sc2)[:, None] * v
                Wm[blk, blk] = m
                Wv[blk, blk] = v
    wmean = _fp22_round(wmean)
    wvar = _fp22_round(wvar)

    # --- reduction vectors (G, R, 2): col0 mean ones, col1 var 1/pp ---
    redw = np.zeros((G, R, 2), np.float64)
    for g in range(G):
        for c in range(CPG):
            p = g * CPG + c
            if p >= P:
                continue
            blk = slice(K * c, K * c + K)
            redw[g, blk, 0] = 1.0
            redw[g, blk, 1] = 1.0 / post_prec[p]
    redw = _fp22_round(redw)

    # --- exp biases: log binom / 2 log binom (per partition) ----------
    logb = np.log(np.array([comb(ORDER, k) for k in range(K)], np.float64))
    comp = _STATE_TRUNC_COMP if int(os.environ.get("BB_BIASCOMP", "1")) else 0.0
    biasv = np.zeros((R, 2), np.float64)
    biasv[:, 0] = np.tile(logb, CPG) + comp
    biasv[:, 1] = 2.0 * np.tile(logb, CPG) + comp
    biasv = biasv.astype(np.float32)

    shared = dict(amat=amat, wmean=wmean, wvar=wvar, redw=redw, biasv=biasv)
    return uv_shards, shared


def _build_module(nloc=NLOC):
    nchunk = nloc // CHUNK if nloc >= CHUNK else 1
    chunk = min(CHUNK, nloc)
    nred = nloc // HALF if nloc >= HALF else 1
    rhalf = min(HALF, nloc)

    nc = bacc.Bacc("TRN2", target_bir_lowering=False, debug=False)
    uv_d = nc.dram_tensor("uv", [4 * D, nloc], F32R, kind="ExternalInput").ap()
    amat_d = nc.dram_tensor("amat", [D * G, 4 * D, R], F32R, kind="ExternalInput").ap()
    wm_d = nc.dram_tensor("wmean", [(D - 1) * G, R, R], F32R, kind="ExternalInput").ap()
    wv_d = nc.dram_tensor("wvar", [(D - 1) * G, R, R], F32R, kind="ExternalInput").ap()
    red_d = nc.dram_tensor("redw", [G, R, 2], F32R, kind="ExternalInput").ap()
    bias_d = nc.dram_tensor("biasv", [R, 2], F32, kind="ExternalInput").ap()
    out_d = nc.dram_tensor("out", [2, nloc], F32, kind="ExternalOutput").ap()

    with tile.TileContext(nc) as tc:
        with (
            tc.tile_pool(name="persist", bufs=1) as persist,
            tc.tile_pool(name="wpool", bufs=4) as wpool,
            tc.tile_pool(name="mpool", bufs=3) as mpool,
            tc.tile_pool(name="psA", bufs=2, space="PSUM") as psA,
            tc.tile_pool(name="psB", bufs=1, space="PSUM") as psB,
        ):
            uv = persist.tile([4 * D, nloc], F32R, tag="uv")
            nc.sync.dma_start(uv[:], uv_d)
            bias = persist.tile([R, 2], F32, tag="bias")
            nc.sync.dma_start(bias[:], bias_d)
            states = []
            for g in range(G):
                s = persist.tile([R, 2, nloc], F32R, tag=f"S{g}")
                states.append(s)
            redt = []
            for g in range(G):
                r = persist.tile([R, 2], F32R, tag=f"RW{g}")
                nc.sync.dma_start(r[:], red_d[g])
                redt.append(r)

            tilecnt = 0
            for d in range(D):
                for g in range(G):
                    a_t = wpool.tile([4 * D, R], F32R, tag="A")
                    nc.sync.dma_start(a_t[:], amat_d[d * G + g])
                    if d >= 1:
                        wm_t = wpool.tile([R, R], F32R, tag="WM")
                        nc.sync.dma_start(wm_t[:], wm_d[(d - 1) * G + g])
                        wv_t = wpool.tile([R, R], F32R, tag="WV")
                        nc.sync.dma_start(wv_t[:], wv_d[(d - 1) * G + g])
                    S = states[g]
                    for ci in range(nchunk):
                        c0 = ci * chunk
                        lm = psA.tile([R, chunk], F32, tag="logM")
                        for h in range(chunk // rhalf):
                            nc.tensor.matmul(
                                lm[:, h * rhalf:(h + 1) * rhalf],
                                a_t[:],
                                uv[:, c0 + h * rhalf:c0 + (h + 1) * rhalf],
                                start=True,
                                stop=True,
                            )
                        if d == 0:
                            # initial states are the multipliers themselves
                            # (meanw0 / varw0 prefactors folded into d=1 weights)
                            nc.scalar.activation(
                                S[:, 0, c0:c0 + chunk], lm[:], EXP,
                                bias=bias[:, 0:1], scale=1.0)
                            nc.scalar.activation(
                                S[:, 1, c0:c0 + chunk], lm[:], EXP,
                                bias=bias[:, 1:2], scale=2.0)
                        else:
                            m_t = mpool.tile([R, 2, chunk], F32, tag="M")
                            nc.scalar.activation(
                                m_t[:, 0, :], lm[:], EXP,
                                bias=bias[:, 0:1], scale=1.0)
                            if tilecnt % GP_EVERY == 0:
                                nc.gpsimd.tensor_tensor(
                                    m_t[:, 1, :], m_t[:, 0, :], m_t[:, 0, :], MULT)
                            else:
                                nc.scalar.activation(
                                    m_t[:, 1, :], lm[:], EXP,
                                    bias=bias[:, 1:2], scale=2.0)
                            tilecnt += 1
                            ch = psB.tile([R, 2, chunk], F32, tag="CH")
                            for h in range(chunk // rhalf):
                                nc.tensor.matmul(
                                    ch[:, 0, h * rhalf:(h + 1) * rhalf],
                                    wm_t[:],
                                    S[:, 0, c0 + h * rhalf:c0 + (h + 1) * rhalf],
                                    start=True,
                                    stop=True,
                                )
                            for h in range(chunk // rhalf):
                                nc.tensor.matmul(
                                    ch[:, 1, h * rhalf:(h + 1) * rhalf],
                                    wv_t[:],
                                    S[:, 1, c0 + h * rhalf:c0 + (h + 1) * rhalf],
                                    start=True,
                                    stop=True,
                                )
                            nc.vector.tensor_tensor(
                                S[:, :, c0:c0 + chunk], ch[:], m_t[:], MULT)

            # ---- final reduction: sum over (chain, k) partitions -----
            # single partition row: [mean(nloc) | var(nloc)] (engine APs
            # must start on quadrant-aligned partitions, so no row 1)
            outs = persist.tile([1, 2 * nloc], F32, tag="outs")
            for ci in range(nred):
                o0 = ci * rhalf
                pm = psA.tile([1, rhalf], F32, tag="logM")
                pv = psB.tile([1, rhalf], F32, tag="CH")
                for g in range(G):
                    nc.tensor.matmul(
                        pm[:], redt[g][:, 0:1],
                        states[g][:, 0, o0:o0 + rhalf],
                        start=(g == 0), stop=(g == G - 1))
                for g in range(G):
                    nc.tensor.matmul(
                        pv[:], redt[g][:, 1:2],
                        states[g][:, 1, o0:o0 + rhalf],
                        start=(g == 0), stop=(g == G - 1))
                nc.vector.tensor_copy(outs[0:1, o0:o0 + rhalf], pm[:])
                nc.vector.tensor_copy(outs[0:1, nloc + o0:nloc + o0 + rhalf], pv[:])
            nc.sync.dma_start(out_d.rearrange("a b -> (a b)")[None, :], outs[:])

    nc.compile()
    return nc


def kernel(Xnew, meanw0, meanw, varw0, varw, prior_sc, post_prec, perm):
    uv_shards, shared = _host_tensors(
        Xnew, meanw0, meanw, varw0, varw, prior_sc, post_prec, perm)
    nc = _build_module(NLOC)
    in_maps = [dict(uv=uv_shards[i], **shared) for i in range(NCORES)]
    res = bass_utils.run_bass_kernel_spmd(
        nc, in_maps, core_ids=list(range(NCORES)))
    outs = [res.results[i]["out"] for i in range(NCORES)]
    f_mean = np.concatenate([o[0] for o in outs]).reshape(N, 1).astype(np.float32)
    f_var = np.concatenate([o[1] for o in outs]).reshape(N, 1).astype(np.float32)
    return f_mean, f_var


# revision 8
# speedup vs baseline: 1.1980x; 1.1980x over previous
"""Trainium2 Bass kernel for nn_BezierButtress (Bernstein-basis permutation chains).

Math (per permutation chain p, over depth d = 0..31):
    S_mean <- (S_mean @ Wm_d) * B(x_{perm[p,d]})        (K=17 wide state)
    S_var  <- (S_var  @ Wv_d) * B(x_{perm[p,d]})^2
    outputs: f_mean[n] = sum_{p,k} S_mean, f_var[n] = sum_{p,k} S_var / post_prec[p]

Device strategy (data-parallel over N across 8 cores, 3072 rows each):
  * state layout: (7 chains x 17 k = 119 partitions, n free), block-diagonal
    119x119 fp32r chain matmuls (3 groups cover 20 chains).
  * per-step Bernstein multipliers built in log space: one PE matmul contracts
    a baked selection/coefficient matrix A_{d,g} (128 x 119) against a resident
    log-table UV (U_hi/V_hi/U_lo/V_lo, 128 x n) giving
    logM = k*log(x_c) + (16-k)*log(1-x_c) exactly (hi/lo splitting cancels the
    PE fp22 truncation); then ACT computes exp(logM + log binom) and
    exp(2*logM + 2*log binom) (the squared multiplier) -- GPSIMD computes the
    square for half the tiles to offload ACT.
  * meanw0 / exp(varw0)*sc2 / sc2 column scale / 1/post_prec are all folded
    host-side into the baked block-diagonal weights & reduction vectors.
"""

import os
import numpy as np
from math import comb

import concourse.bass as bass
import concourse.mybir as mybir
import concourse.tile as tile
from concourse import bacc
from concourse import bass_utils

ORDER = 16
K = 17
D = 32
P = 20
N = 24576
NCORES = 8
NLOC = N // NCORES        # 3072
CPG = 7                   # chain slots per group
G = 3                     # groups (7, 7, 6 + 1 pad)
R = CPG * K               # 119 partitions
CHUNK = 1024
HALF = 512
F32 = mybir.dt.float32
F32R = mybir.dt.float32r
EXP = mybir.ActivationFunctionType.Exp
MULT = mybir.AluOpType.mult

# fraction of squared-multiplier tiles computed on GPSIMD instead of ACT
GP_EVERY = 2  # every 2nd tile -> 50%


def _fp22_split(x64):
    """Split float64 -> (hi, lo) float32 with hi exactly representable in
    fp22 (e10m11), so PE fp32r matmuls consume hi/lo exactly."""
    x32 = x64.astype(np.float32)
    hi = (x32.view(np.uint32) & np.uint32(0xFFFFF000)).view(np.float32)
    lo = (x64 - hi.astype(np.float64)).astype(np.float32)
    return hi, lo


def _fp22_round(x64):
    """Round float64 to the nearest fp22 (e10m11) value, returned as float32.
    The PE's fp32r path *truncates* inputs to fp22; feeding it pre-rounded
    values makes that truncation a no-op and kills the systematic bias."""
    x32 = x64.astype(np.float32)
    u = x32.view(np.uint32).astype(np.uint64)
    u = ((u + 0x800) & 0xFFFFF000).astype(np.uint32)   # round-half-up on m11
    return u.view(np.float32)


# Each chain step's state is stored as fp32 and truncated to fp22 (e10m11)
# when the PE streams it: expected relative loss ~ln2*2^-12 per step.
# Compensate by scaling every per-step multiplier accordingly
# (opt-out: BB_BIASCOMP=0).
_STATE_TRUNC_COMP = float(np.log1p(np.log(2.0) * 2.0 ** -12))


def _host_tensors(Xnew, meanw0, meanw, varw0, varw, prior_sc, post_prec, perm):
    Xnew = np.asarray(Xnew, np.float32)
    meanw0 = np.asarray(meanw0, np.float64)   # (P, 1, K)
    meanw = np.asarray(meanw, np.float64)     # (D-1, P, K, K)
    varw0 = np.asarray(varw0, np.float64)     # (P, 1, K)
    varw = np.asarray(varw, np.float64)       # (D-1, P, K, K)
    prior_sc = np.asarray(prior_sc, np.float64)  # (K, 1)
    post_prec = np.asarray(post_prec, np.float64)  # (P,)
    perm = np.asarray(perm)                   # (P, D) int

    # --- per-core UV log tables ---------------------------------------
    x64 = np.clip(Xnew.astype(np.float64), 1e-30, None)
    u64 = np.log(x64)                                    # (N, D)
    v64 = np.log1p(-np.minimum(Xnew.astype(np.float64), 1.0 - 1e-15))
    uh, ul = _fp22_split(u64)
    vh, vl = _fp22_split(v64)
    uv_full = np.concatenate(
        [uh.T[None], vh.T[None], ul.T[None], vl.T[None]], axis=0
    )  # (4, D, N)
    uv_shards = []
    for i in range(NCORES):
        sl = uv_full[:, :, i * NLOC:(i + 1) * NLOC]      # (4, D, NLOC)
        uv_shards.append(np.ascontiguousarray(sl.reshape(4 * D, NLOC), np.float32))

    # --- A selection/coefficient matrices (D*G, 128, R) ---------------
    ks = np.arange(K, dtype=np.float64)
    amat = np.zeros((D * G, 4 * D, R), np.float64)
    for d in range(D):
        for g in range(G):
            A = amat[d * G + g]
            for c in range(CPG):
                p = g * CPG + c
                if p >= P:
                    continue
                col = perm[p, d]
                j = slice(K * c, K * c + K)
                A[col, j] = ks
                A[D + col, j] = ORDER - ks
                A[2 * D + col, j] = ks
                A[3 * D + col, j] = ORDER - ks
    amat = amat.astype(np.float32)

    # --- block-diagonal chain weights ---------------------------------
    sc2 = prior_sc[:, 0] ** 2                            # (K,)
    wmean = np.zeros(((D - 1) * G, R, R), np.float64)
    wvar = np.zeros(((D - 1) * G, R, R), np.float64)
    for d in range(1, D):
        for g in range(G):
            Wm = wmean[(d - 1) * G + g]
            Wv = wvar[(d - 1) * G + g]
            for c in range(CPG):
                p = g * CPG + c
                if p >= P:
                    continue
                blk = slice(K * c, K * c + K)
                m = meanw[d - 1, p]                      # (K, K) [k, j]
                v = np.exp(varw[d - 1, p]) * sc2[None, :]
                if d == 1:
                    m = meanw0[p, 0][:, None] * m
                    v = (np.exp(varw0[p, 0]) * sc2)[:, None] * v
                Wm[blk, blk] = m
                Wv[blk, blk] = v
    wmean = _fp22_round(wmean)
    wvar = _fp22_round(wvar)

    # --- reduction vectors (G, R, 2): col0 mean ones, col1 var 1/pp ---
    # factor the geometric-mean scale of 1/post_prec out to the host so the
    # device-side values are ~1 (exactly 1 for uniform post_prec: no rounding)
    qbar = float(np.exp(np.mean(np.log(1.0 / post_prec))))
    qbar_inv = (1.0 / post_prec) / qbar
    redw = np.zeros((G, R, 2), np.float64)
    for g in range(G):
        for c in range(CPG):
            p = g * CPG + c
            if p >= P:
                continue
            blk = slice(K * c, K * c + K)
            redw[g, blk, 0] = 1.0
            redw[g, blk, 1] = qbar_inv[p]
    redw = _fp22_round(redw)

    # --- exp biases: log binom / 2 log binom (per partition) ----------
    logb = np.log(np.array([comb(ORDER, k) for k in range(K)], np.float64))
    comp = _STATE_TRUNC_COMP if int(os.environ.get("BB_BIASCOMP", "1")) else 0.0
    biasv = np.zeros((R, 2), np.float64)
    biasv[:, 0] = np.tile(logb, CPG) + comp
    biasv[:, 1] = 2.0 * np.tile(logb, CPG) + comp
    biasv = biasv.astype(np.float32)

    shared = dict(amat=amat, wmean=wmean, wvar=wvar, redw=redw, biasv=biasv)
    return uv_shards, shared, qbar


def _build_module(nloc=NLOC):
    nchunk = nloc // CHUNK if nloc >= CHUNK else 1
    chunk = min(CHUNK, nloc)
    nred = nloc // HALF if nloc >= HALF else 1
    rhalf = min(HALF, nloc)

    nc = bacc.Bacc("TRN2", target_bir_lowering=False, debug=False)
    uv_d = nc.dram_tensor("uv", [4 * D, nloc], F32R, kind="ExternalInput").ap()
    amat_d = nc.dram_tensor("amat", [D * G, 4 * D, R], F32R, kind="ExternalInput").ap()
    wm_d = nc.dram_tensor("wmean", [(D - 1) * G, R, R], F32R, kind="ExternalInput").ap()
    wv_d = nc.dram_tensor("wvar", [(D - 1) * G, R, R], F32R, kind="ExternalInput").ap()
    red_d = nc.dram_tensor("redw", [G, R, 2], F32R, kind="ExternalInput").ap()
    bias_d = nc.dram_tensor("biasv", [R, 2], F32, kind="ExternalInput").ap()
    out_d = nc.dram_tensor("out", [2, nloc], F32, kind="ExternalOutput").ap()

    with tile.TileContext(nc) as tc:
        with (
            tc.tile_pool(name="persist", bufs=1) as persist,
            tc.tile_pool(name="wpool", bufs=4) as wpool,
            tc.tile_pool(name="mpool", bufs=3) as mpool,
            tc.tile_pool(name="psA", bufs=2, space="PSUM") as psA,
            tc.tile_pool(name="psB", bufs=1, space="PSUM") as psB,
        ):
            uv = persist.tile([4 * D, nloc], F32R, tag="uv")
            nc.sync.dma_start(uv[:], uv_d)
            bias = persist.tile([R, 2], F32, tag="bias")
            nc.sync.dma_start(bias[:], bias_d)
            states = []
            for g in range(G):
                s = persist.tile([R, 2, nloc], F32R, tag=f"S{g}")
                states.append(s)
            redt = []
            for g in range(G):
                r = persist.tile([R, 2], F32R, tag=f"RW{g}")
                nc.sync.dma_start(r[:], red_d[g])
                redt.append(r)

            tilecnt = 0
            for d in range(D):
                for g in range(G):
                    a_t = wpool.tile([4 * D, R], F32R, tag="A")
                    nc.sync.dma_start(a_t[:], amat_d[d * G + g])
                    if d >= 1:
                        wm_t = wpool.tile([R, R], F32R, tag="WM")
                        nc.sync.dma_start(wm_t[:], wm_d[(d - 1) * G + g])
                        wv_t = wpool.tile([R, R], F32R, tag="WV")
                        nc.sync.dma_start(wv_t[:], wv_d[(d - 1) * G + g])
                    S = states[g]
                    for ci in range(nchunk):
                        c0 = ci * chunk
                        lm = psA.tile([R, chunk], F32, tag="logM")
                        for h in range(chunk // rhalf):
                            nc.tensor.matmul(
                                lm[:, h * rhalf:(h + 1) * rhalf],
                                a_t[:],
                                uv[:, c0 + h * rhalf:c0 + (h + 1) * rhalf],
                                start=True,
                                stop=True,
                            )
                        if d == 0:
                            # initial states are the multipliers themselves
                            # (meanw0 / varw0 prefactors folded into d=1 weights)
                            nc.scalar.activation(
                                S[:, 0, c0:c0 + chunk], lm[:], EXP,
                                bias=bias[:, 0:1], scale=1.0)
                            nc.scalar.activation(
                                S[:, 1, c0:c0 + chunk], lm[:], EXP,
                                bias=bias[:, 1:2], scale=2.0)
                        else:
                            m_t = mpool.tile([R, 2, chunk], F32, tag="M")
                            nc.scalar.activation(
                                m_t[:, 0, :], lm[:], EXP,
                                bias=bias[:, 0:1], scale=1.0)
                            if tilecnt % GP_EVERY == 0:
                                nc.gpsimd.tensor_tensor(
                                    m_t[:, 1, :], m_t[:, 0, :], m_t[:, 0, :], MULT)
                            else:
                                nc.scalar.activation(
                                    m_t[:, 1, :], lm[:], EXP,
                                    bias=bias[:, 1:2], scale=2.0)
                            tilecnt += 1
                            ch = psB.tile([R, 2, chunk], F32, tag="CH")
                            for h in range(chunk // rhalf):
                                nc.tensor.matmul(
                                    ch[:, 0, h * rhalf:(h + 1) * rhalf],
                                    wm_t[:],
                                    S[:, 0, c0 + h * rhalf:c0 + (h + 1) * rhalf],
                                    start=True,
                                    stop=True,
                                )
                            for h in range(chunk // rhalf):
                                nc.tensor.matmul(
                                    ch[:, 1, h * rhalf:(h + 1) * rhalf],
                                    wv_t[:],
                                    S[:, 1, c0 + h * rhalf:c0 + (h + 1) * rhalf],
                                    start=True,
                                    stop=True,
                                )
                            nc.vector.tensor_tensor(
                                S[:, :, c0:c0 + chunk], ch[:], m_t[:], MULT)

            # ---- final reduction: sum over (chain, k) partitions -----
            # single partition row: [mean(nloc) | var(nloc)] (engine APs
            # must start on quadrant-aligned partitions, so no row 1)
            outs = persist.tile([1, 2 * nloc], F32, tag="outs")
            for ci in range(nred):
                o0 = ci * rhalf
                pm = psA.tile([1, rhalf], F32, tag="logM")
                pv = psB.tile([1, rhalf], F32, tag="CH")
                for g in range(G):
                    nc.tensor.matmul(
                        pm[:], redt[g][:, 0:1],
                        states[g][:, 0, o0:o0 + rhalf],
                        start=(g == 0), stop=(g == G - 1))
                for g in range(G):
                    nc.tensor.matmul(
                        pv[:], redt[g][:, 1:2],
                        states[g][:, 1, o0:o0 + rhalf],
                        start=(g == 0), stop=(g == G - 1))
                nc.vector.tensor_copy(outs[0:1, o0:o0 + rhalf], pm[:])
                nc.vector.tensor_copy(outs[0:1, nloc + o0:nloc + o0 + rhalf], pv[:])
            nc.sync.dma_start(out_d.rearrange("a b -> (a b)")[None, :], outs[:])

    nc.compile()
    return nc


def kernel(Xnew, meanw0, meanw, varw0, varw, prior_sc, post_prec, perm):
    uv_shards, shared, qbar = _host_tensors(
        Xnew, meanw0, meanw, varw0, varw, prior_sc, post_prec, perm)
    nc = _build_module(NLOC)
    in_maps = [dict(uv=uv_shards[i], **shared) for i in range(NCORES)]
    res = bass_utils.run_bass_kernel_spmd(
        nc, in_maps, core_ids=list(range(NCORES)))
    outs = [res.results[i]["out"] for i in range(NCORES)]
    f_mean = np.concatenate([o[0] for o in outs]).reshape(N, 1).astype(np.float32)
    f_var = (np.concatenate([o[1] for o in outs]).reshape(N, 1)
             * np.float32(qbar)).astype(np.float32)
    return f_mean, f_var
